# revision 76
# baseline (speedup 1.0000x reference)
"""Bahdanau (additive) attention kernel for Trainium2, 8-core data-parallel.

Problem (per reference):
    q = query @ Wq                                  [B,Tq,U]
    k = value @ Wk                                  [B,Tk,U]
    scores[b,i,j] = sum_u scale[u] * tanh(q[b,i,u] + k[b,j,u])
    scores = where(mask, scores, -1e9)
    attn = softmax(scores, axis=-1)                 [B,Tq,Tk]
    ctx  = attn @ value                             [B,Tq,D]
    returns (ctx, attn)

Shapes: B=4, Tq=Tk=256, D=1024, U=512, fp32.

Sharding: 8 cores, core c handles batch b=c//2, query half h=c%2 (128
queries). Fully data-parallel, no collectives.

Per-core device algorithm (layout trick: U on partitions so the q+k
broadcast-add becomes a per-partition `tensor_scalar`):
  1. PE: per u-chunk c (128 of U), kT_c[u,j] = Wk_c.T @ value.T and
     qT_c[u,i] = Wq_c.T @ query.T (bf16 operands, fp32 PSUM), both into
     one PSUM bank, one ACT copy out to SBUF.
  2. Main loop over blocks of G=8 queries:
     DVE: S[u, (g,c,j)] = kT_c[u,j] + qT_c[u,i]  (tensor_scalar add,
          2x mode, per-partition scalar = qT column)
     ACT: T = tanh(S) as one [128, 8192] instruction (the ~109us/core
          ScalarE floor that bounds this kernel)
     PE:  scores[i,j] = sum_u scale_u * T via M=32 matvecs whose lhsT
          has scale embedded in column r and zeros elsewhere, so each
          accumulates PSUM row r of [32,1024] and adds exact zeros to
          the rest (matmul PSUM outputs must start at partition
          0/32/64, so an M=1 matvec cannot address row r directly).
          Row r holds queries i=4r+g, g=0..3.
  3. Mask folded in by initializing score PSUM with ones(32,1)@maskrep.
  4. Softmax without max-subtraction (|scores| <= sum|scale| ~ 18, exp
     is safe in fp32; masked lanes give exp(-1e9)=0 exactly).
  5. PE: attn.T via identity transposes, ctx = fp32r matmuls vs value.
"""

import os

import numpy as np

B, TQ, TK, D, U = 4, 256, 256, 1024, 512
NCORES = 8
HQ = 128          # queries per core
G = 8             # queries per tanh block
NBLK = HQ // G    # 16
NCHUNK = U // 128  # 4
NEG = -1e9

# stash of the last BassKernelResults (exec_time_ns etc. when BASS_TRACE=1)
LAST_RUN = None

_PROGRAM_CACHE = {}


def _split_excess_waits(nc):
    """walrus in this container rejects engine instructions carrying more
    than one sync wait. Hoist extra waits onto same-engine NOPs inserted
    right before the offending instruction (the engine stalls on the NOPs
    first, so the semantics are identical)."""
    import concourse.mybir as mybir

    n_new = 0
    for fn in nc.m.functions:
        for bb in fn.blocks:
            insts = bb.instructions
            i = 0
            while i < len(insts):
                inst = insts[i]
                si = getattr(inst, "sync_info", None)
                if si is not None and si.on_wait and len(si.on_wait) > 1:
                    waits = list(si.on_wait)
                    inst.sync_info = mybir.SyncInfo(
                        on_wait=[waits[-1]], on_update=list(si.on_update or [])
                    )
                    for w in waits[:-1]:
                        n_new += 1
                        nop = mybir.InstNoOp(name=f"WS-{n_new}-{inst.name}",
                                             ins=[], outs=[])
                        nop.engine = inst.engine
                        nop.sync_info = mybir.SyncInfo(on_wait=[w],
                                                       on_update=[])
                        insts.insert(i, nop)
                        i += 1
                i += 1
    return n_new


def _install_drain_patch():
    """Kept for API compatibility (hwtime.py calls it); the generic
    _split_excess_waits post-pass now handles the tail drain too."""


def _build_program(repeats=1, split_waits=True):
    from contextlib import ExitStack

    import concourse.bass as bass
    import concourse.mybir as mybir
    import concourse.tile as tile

    _install_drain_patch()

    dt = mybir.dt
    AF = mybir.ActivationFunctionType
    f32, f32r, bf16 = dt.float32, dt.float32r, dt.bfloat16

    nc = bass.Bass("TRN2", target_bir_lowering=False, debug=False)

    # ---- DRAM I/O (per-core data, identical program) ----
    vqT = nc.dram_tensor("vqT", [D, TK + HQ], bf16, kind="ExternalInput").ap()
    v_in = nc.dram_tensor("v", [TK, D], f32r, kind="ExternalInput").ap()
    wq = nc.dram_tensor("wq", [D, U], bf16, kind="ExternalInput").ap()
    wk = nc.dram_tensor("wk", [D, U], bf16, kind="ExternalInput").ap()
    # scale_t[:, c] holds scale chunk c; expanded on-device into scale_diag,
    # whose [128, 32] slice at c*1024+r*32 has scale chunk c in column r and
    # zeros elsewhere: the M=32 matvec then accumulates row r of the [32, 512]
    # score PSUM and adds exact zeros to the other 31 rows (matmul PSUM
    # outputs must start at partition 0/32/64, so an M=1 matvec cannot
    # address row r directly).
    # misc: cols 0-3 scale_t, cols 4-131 identity(128)
    misc = nc.dram_tensor("misc", [128, NCHUNK + 128], f32,
                          kind="ExternalInput").ap()
    # maskones: cols 0-1023 maskrep, cols 1024-1055 ones
    maskones = nc.dram_tensor("maskones", [1, 4 * TK + 32], f32r,
                              kind="ExternalInput").ap()
    ones128 = nc.dram_tensor("ones128", [128, 1], f32,
                             kind="ExternalInput").ap()

    ctx_out = nc.dram_tensor("ctx", [HQ, D], f32, kind="ExternalOutput").ap()
    attn_out = nc.dram_tensor("attn", [HQ // 4, 4 * TK], f32,
                              kind="ExternalOutput").ap()

    KQW = TK + HQ  # 384, width of one (kT | qT) chunk group in kqT_sb

    with tile.TileContext(nc) as tc:
        with ExitStack() as stack:
            const_pool = stack.enter_context(tc.tile_pool(name="const", bufs=1))
            work_pool = stack.enter_context(tc.tile_pool(name="work", bufs=1))
            pproj_pool = stack.enter_context(
                tc.tile_pool(name="pproj", bufs=2, space="PSUM"))
            pt_pool = stack.enter_context(
                tc.tile_pool(name="pt", bufs=2, space="PSUM"))
            pscore_pool = stack.enter_context(
                tc.tile_pool(name="pscore", bufs=1, space="PSUM"))
            pctx_pool = stack.enter_context(
                tc.tile_pool(name="pctx", bufs=2, space="PSUM"))
            # ---- constants ----
            maskones_sb = const_pool.tile([1, 4 * TK + 32], f32r, tag="maskon")
            nc.sync.dma_start(maskones_sb[:], maskones[:])
            ones_sb = maskones_sb[:1, 4 * TK: 4 * TK + 32]
            # dummy tanh pulls the ~2.7us ACT table load off the critical path
            warm_sb = const_pool.tile([1, 32], f32, tag="warm")
            nc.scalar.activation(warm_sb[:], ones_sb, AF.Tanh)
            misc_sb = const_pool.tile([128, NCHUNK + 128], f32, tag="misc")
            nc.sync.dma_start(misc_sb[:], misc[:])
            ones128_sb = const_pool.tile([128, 1], f32, tag="ones128")
            scale_t_sb = misc_sb[:, 0:NCHUNK]
            ident_sb = misc_sb[:, NCHUNK: NCHUNK + 128]
            # PE warmup: keep TensorE busy before the projection matmuls
            # so the HAM clock gate opens (P3). Uses the same f32r
            # rank-1 pattern as the mask init (an f32 lhsT==rhs variant
            # crashed the exec unit).
            wps = pproj_pool.tile([128, 512], f32, tag="proj")
            for wi in range(16):
                nc.tensor.matmul(wps[0:32, :],
                                 lhsT=ones_sb,
                                 rhs=maskones_sb[:1, 0:512],
                                 start=(wi == 0), stop=(wi == 15),
                                 skip_group_check=True)
            scale_sb = const_pool.tile([128, NCHUNK * 1024], bf16, tag="scale")
            nc.vector.memset(scale_sb[:], 0.0)
            for c in range(NCHUNK):
                # place scale chunk c on the diagonal: cols c*1024 + r*33
                nc.vector.tensor_copy(
                    scale_sb[:, c * 1024: (c + 1) * 1024: 33],
                    scale_t_sb[:, c: c + 1].broadcast_to((128, 32)),
                )
            # kqT_sb chunk c: cols [c*KQW, c*KQW+TK) = kT_c[u, j],
            #                 cols [c*KQW+TK, (c+1)*KQW) = qT_c[u, i]
            kqT_sb = work_pool.tile([128, NCHUNK * KQW], f32, tag="kqT")
            # value tiles (the DMA is issued later, after the weight DMAs)
            v_sb = []
            for jt in range(2):
                v_tile = const_pool.tile([128, D], f32r, tag=f"v{jt}")
                v_sb.append(v_tile)

            # S pool sits on fresh addresses (NOT reusing the weights pool)
            # so the first S adds don't wait for the whole projection phase
            s_pool = stack.enter_context(tc.tile_pool(name="sblk", bufs=2))

            # ---- projections (weights pool scoped: 32KB/partition of Wq/Wk
            # plus vqT are freed before the S/T block pools open) ----
            with tc.tile_pool(name="wts", bufs=1) as w_pool:
                # one consolidated DMA per tensor half: DMA issue is the
                # serial resource in the head, not bandwidth
                # ~0.25MB pieces: few enough that per-DMA issue overhead is
                # small, many enough to spread across the HW DMA queues
                vqT_sb = w_pool.tile([128, 8 * KQW], bf16, tag="vqT")
                vqT_r = vqT.rearrange("(kt p) w -> p kt w", p=128)
                vqT_s = vqT_sb[:].rearrange("p (kt w) -> p kt w", kt=8)
                wq_sb = w_pool.tile([128, 8 * U], bf16, tag="wq")
                wk_sb = w_pool.tile([128, 8 * U], bf16, tag="wk")
                wq4 = wq.rearrange("(kt p) u -> p kt u", p=128)
                wk4 = wk.rearrange("(kt p) u -> p kt u", p=128)
                wq_s = wq_sb[:].rearrange("p (kt u) -> p kt u", kt=8)
                wk_s = wk_sb[:].rearrange("p (kt u) -> p kt u", kt=8)
                for quarter in range(4):
                    sl = slice(quarter * 2, (quarter + 1) * 2)
                    nc.sync.dma_start(vqT_s[:, sl], vqT_r[:, sl])
                # chunk-major halves: chunk pair {c, c+1}'s projections start
                # once their weight columns land (256-col pieces keep DMA
                # bursts at 512B)
                for cp in range(2):
                    cs = slice(cp * 256, (cp + 1) * 256)
                    nc.sync.dma_start(wk_s[:, 0:4, cs], wk4[:, 0:4, cs])
                    nc.sync.dma_start(wk_s[:, 4:8, cs], wk4[:, 4:8, cs])
                    nc.sync.dma_start(wq_s[:, 0:4, cs], wq4[:, 0:4, cs])
                    nc.sync.dma_start(wq_s[:, 4:8, cs], wq4[:, 4:8, cs])

                # per chunk c: kT[u,j] = Wk_c.T @ value.T into psum cols
                # 0:256, qT[u,i] = Wq_c.T @ query.T into cols 256:384 of the
                # same bank, then one ACT copy moves both into kqT_sb
                for cp in range(2):
                    kps_pair = []
                    for ci in range(2):
                        kps = pproj_pool.tile([128, 512], f32, tag="proj")
                        kps_pair.append(kps)
                    # interleave the pair's chunks so every arriving weight
                    # piece feeds back-to-back matmuls
                    for kt in range(8):
                        for ci in range(2):
                            c = cp * 2 + ci
                            nc.tensor.matmul(
                                kps_pair[ci][:, 0:TK],
                                lhsT=wk_sb[:, kt * U + c * 128:
                                           kt * U + (c + 1) * 128],
                                rhs=vqT_sb[:, kt * KQW: kt * KQW + TK],
                                start=(kt == 0), stop=(kt == 7),
                                skip_group_check=True,
                            )
                    for kt in range(8):
                        for ci in range(2):
                            c = cp * 2 + ci
                            nc.tensor.matmul(
                                kps_pair[ci][:, TK:KQW],
                                lhsT=wq_sb[:, kt * U + c * 128:
                                           kt * U + (c + 1) * 128],
                                rhs=vqT_sb[:, kt * KQW + TK: (kt + 1) * KQW],
                                start=(kt == 0), stop=(kt == 7),
                                skip_group_check=True,
                            )
                    for ci in range(2):
                        c = cp * 2 + ci
                        nc.scalar.copy(kqT_sb[:, c * KQW: (c + 1) * KQW],
                                       kps_pair[ci][:, 0:KQW])

                # tail-only constants and value: DMA last
                nc.sync.dma_start(ones128_sb[:], ones128[:])
                for jt in range(2):
                    for ph in range(2):
                        nc.sync.dma_start(
                            v_sb[jt][ph * 64:(ph + 1) * 64, :],
                            v_in[jt * 128 + ph * 64:
                                 jt * 128 + (ph + 1) * 64, :])

            # ---- scores PSUM, mask init ----
            scores_ps = pscore_pool.tile([HQ // 4, 4 * TK], f32, tag="scores")
            for half in range(2):
                nc.tensor.matmul(
                    scores_ps[:, half * 512:(half + 1) * 512],
                    lhsT=ones_sb,
                    rhs=maskones_sb[:1, half * 512:(half + 1) * 512],
                    start=True, stop=False, skip_group_check=True,
                )

            # ---- main loop (repeats>1 only for timing amplification) ----
            t_pool = stack.enter_context(tc.tile_pool(name="tblk", bufs=3))
            for blk in range(NBLK * repeats):
                blk = blk % NBLK
                S = s_pool.tile([128, G * 1024], f32, tag="S")
                # c outer: the first block's adds start as soon as kqT chunk
                # c lands, instead of waiting for all four chunks
                for c, g in [(c, g) for c in range(NCHUNK)
                             for g in range(G)]:
                    i_l = blk * G + g
                    nc.vector.tensor_scalar_add(
                        S[:, (c * G + g) * 256:(c * G + g + 1) * 256],
                        kqT_sb[:, c * KQW: c * KQW + TK],
                        kqT_sb[:, c * KQW + TK + i_l:
                                c * KQW + TK + i_l + 1],
                    )
                T = t_pool.tile([128, G * 1024], bf16, tag="T")
                if blk == NBLK - 1:
                    # split the last tanh so the final matvecs start earlier
                    half_w = G * 512
                    nc.scalar.activation(T[:, 0:half_w], S[:, 0:half_w],
                                         AF.Tanh)
                    nc.scalar.activation(T[:, half_w:], S[:, half_w:],
                                         AF.Tanh)
                else:
                    nc.scalar.activation(T[:], S[:], AF.Tanh)

                for q2 in range(G // 4):
                    r = blk * (G // 4) + q2
                    for half in range(2):
                        g0 = q2 * 4 + half * 2
                        for c in range(NCHUNK):
                            # (c, g, j) layout makes this a contiguous 2D rhs
                            nc.tensor.matmul(
                                scores_ps[:, half * 512:(half + 1) * 512],
                                lhsT=scale_sb[:, c * 1024 + r * 32:
                                              c * 1024 + (r + 1) * 32],
                                rhs=T[:, (c * G + g0) * 256:
                                       (c * G + g0 + 2) * 256],
                                start=False, stop=(c == 3),
                                skip_group_check=True,
                            )

            # ---- softmax (no max-subtraction needed; see module doc) ----
            e_sb = work_pool.tile([HQ // 4, 4 * TK], f32, tag="e")
            nc.scalar.activation(e_sb[:], scores_ps[:], AF.Exp)
            # ---- ctx = softmax(scores) @ value, computed as
            # (E @ value) * (1/rowsum(E)): the transposes and ctx matmuls
            # read UNNORMALIZED exp, so they don't wait for the softmax
            # divide; the per-query reciprocal is folded into the final
            # PSUM->SBUF copy as an ACT per-partition scale ----
            attnT = []
            for jt in range(2):
                at = work_pool.tile([128, HQ], f32r, tag=f"attnT{jt}")
                attnT.append(at)
            for jt in range(2):
                pt = pt_pool.tile([128, 128], f32, tag="pt")
                for g2 in range(4):
                    nc.tensor.transpose(
                        pt[:, g2 * 32:(g2 + 1) * 32],
                        e_sb[:, g2 * 256 + jt * 128: g2 * 256 + (jt + 1) * 128],
                        ident_sb[:32, :32],
                    )
                # pt cols are (g2, r); attnT cols are i = 4r + g2
                at_copy = nc.vector.tensor_copy(
                    attnT[jt][:].rearrange("p (r g) -> p r g", g=4),
                    pt[:].rearrange("p (g r) -> p r g", r=32))

            # per-query 1/sum(E) as [128,1]: N=1 matvec of E.T against ones
            ssum_ps = pscore_pool.tile([128, 1], f32, tag="scores")
            for jt in range(2):
                nc.tensor.matmul(ssum_ps[:],
                                 lhsT=attnT[jt][:].bitcast(f32),
                                 rhs=ones128_sb[:],
                                 start=(jt == 0), stop=(jt == 1),
                                 skip_group_check=True)
            rsumi = work_pool.tile([128, 1], f32, tag="rsumi")
            rsumi_rec = nc.vector.reciprocal(rsumi[:], ssum_ps[:])

            ctx_sb = work_pool.tile([128, D], f32, tag="ctxsb")
            for nh in range(2):
                # separate PSUM tiles per half so half 1's matmuls don't
                # serialize behind the ACT copy of half 0
                ctx_ps = pctx_pool.tile([128, 512], f32, tag="ctx")
                for jt in range(2):
                    nc.tensor.matmul(
                        ctx_ps[:],
                        lhsT=attnT[jt][:],
                        rhs=v_sb[jt][:, nh * 512:(nh + 1) * 512],
                        start=(jt == 0), stop=(jt == 1),
                    )
                nc.scalar.mul(ctx_sb[:, nh * 512:(nh + 1) * 512],
                              ctx_ps[:], rsumi[:])
                nc.sync.dma_start(ctx_out[:, nh * 512:(nh + 1) * 512],
                                  ctx_sb[:, nh * 512:(nh + 1) * 512])

            # attention-weights output path (off the ctx critical path,
            # emitted last so the scheduler deprioritizes it on DVE)
            ssum = work_pool.tile([HQ // 4, 4], f32, tag="ssum")
            red = nc.vector.reduce_sum(
                ssum[:], e_sb[:].rearrange("p (g x) -> p g x", g=4),
                axis=mybir.AxisListType.X)
            # scheduling-only edge: keep the big reduce (attn-weights output
            # path) from gap-filling DVE ahead of the ctx-critical copies
            from concourse.tile_rust import add_dep_helper
            add_dep_helper(red.ins, at_copy.ins, sync=False,
                           reason="deprioritize attn-out reduce")
            add_dep_helper(red.ins, rsumi_rec.ins, sync=False,
                           reason="deprioritize attn-out reduce")
            rsum = work_pool.tile([HQ // 4, 4], f32, tag="rsum")
            nc.vector.reciprocal(rsum[:], ssum[:])
            attn_sb = work_pool.tile([HQ // 4, 4 * TK], f32, tag="attn")
            for g2 in range(4):
                nc.vector.tensor_scalar_mul(
                    attn_sb[:, g2 * 256:(g2 + 1) * 256],
                    e_sb[:, g2 * 256:(g2 + 1) * 256],
                    rsum[:, g2:g2 + 1],
                )
            nc.sync.dma_start(attn_out[:], attn_sb[:])


    if split_waits:
        _split_excess_waits(nc)
    return nc


def _get_program(repeats=1):
    key = ("nc", repeats)
    if key not in _PROGRAM_CACHE:
        _PROGRAM_CACHE[key] = _build_program(repeats)
    return _PROGRAM_CACHE[key]


def _make_in_maps(query, value, mask, Wq, Wk, scale):
    import ml_dtypes

    query = np.asarray(query, dtype=np.float32)
    value = np.asarray(value, dtype=np.float32)
    mask = np.asarray(mask)
    Wq = np.asarray(Wq, dtype=np.float32)
    Wk = np.asarray(Wk, dtype=np.float32)
    scale = np.asarray(scale, dtype=np.float32)

    scale_t = np.ascontiguousarray(scale.reshape(NCHUNK, 128).T)  # [128, 4]
    wq_bf = Wq.astype(ml_dtypes.bfloat16)
    wk_bf = Wk.astype(ml_dtypes.bfloat16)
    misc = np.concatenate([scale_t, np.eye(128, dtype=np.float32)],
                          axis=1)  # [128, 132]

    in_maps = []
    for c in range(NCORES):
        b, h = divmod(c, 2)
        qs = query[b, h * HQ:(h + 1) * HQ, :]        # [128, D]
        vqT_np = np.concatenate([value[b].T, qs.T], axis=1).astype(
            ml_dtypes.bfloat16)  # [D, 384]
        maskadd = np.where(mask[b], 0.0, NEG).astype(np.float32)
        maskones = np.concatenate(
            [np.tile(maskadd, 4), np.ones(32, np.float32)])[None, :]
        in_maps.append({
            "vqT": vqT_np,
            "v": np.ascontiguousarray(value[b]),
            "wq": wq_bf,
            "wk": wk_bf,
            "misc": misc,
            "maskones": maskones,
            "ones128": np.ones((128, 1), np.float32),
        })
    return in_maps


def kernel(query, value, mask, Wq, Wk, scale):
    global LAST_RUN
    from concourse.bass_utils import run_bass_kernel_spmd

    nc = _get_program()
    in_maps = _make_in_maps(query, value, mask, Wq, Wk, scale)
    trace = bool(int(os.environ.get("BAHDANAU_TRACE", "0")))
    try:
        res = run_bass_kernel_spmd(nc, in_maps, list(range(NCORES)),
                                   trace=trace)
    except Exception:
        if not trace:
            raise
        # NTFF profiling hook unavailable (e.g. bare axon container):
        # rerun without tracing
        res = run_bass_kernel_spmd(nc, in_maps, list(range(NCORES)),
                                   trace=False)
    LAST_RUN = res

    ctx = np.empty((B, TQ, D), dtype=np.float32)
    attn = np.empty((B, TQ, TK), dtype=np.float32)
    for c in range(NCORES):
        b, h = divmod(c, 2)
        ctx[b, h * HQ:(h + 1) * HQ] = res.results[c]["ctx"]
        attn[b, h * HQ:(h + 1) * HQ] = res.results[c]["attn"].reshape(HQ, TK)
    return ctx, attn
